# revision 12
# baseline (speedup 1.0000x reference)
"""Trainium2 Bass kernel for nn_DelayLIFSNN.

Architecture (per reference):
  x (B, T0, J) -> delay_conv(w0,p0) -> BN(global batch stats) -> LIF
               -> delay_conv(w1,p1) -> BN -> LIF
               -> delay_conv(wr,pr) -> LI readout -> sum_t softmax_o -> (B, O)

Sharding: data-parallel over batch B across 8 cores (B_loc=32/core);
BN stats all-reduced ((128, 2*HT) f32 = 4KB each).

Host<->device traffic is the wall-clock bottleneck (axon tunnel ~90MB/s),
so the host ships only:
  - x as float16 (B, T0, J)            (~21.5 MB total)
  - one packed weight matrix wcat [H, 1348] f32, SHARDED row-wise across
    cores (~2.7 MB total); an on-device AllGather reassembles it and the
    DCLS gaussian-interpolated conv kernels are computed on device.
The jitted PJRT executable is cached in a module global so repeat calls
don't re-trace.

Compute:
  Conv = sum over K=25 taps of shifted matmuls accumulated in PSUM
  (conv1 in f16, conv2/readout in f32).
  LIF = per-step scalar_tensor_tensor ops on DVE (sequential over time).
  LI readout = tensor_tensor_scan. Softmax+time-sum via PE transpose +
  selector-matmul.

Activation layouts:
  x / spikes (conv rhs): [ch_tile][ch_part 128, t*B + b]
  conv out psum:         [out_part 128, t*B + b] per (ht, time-tile)
  y DRAM:                [HT, 128, T, B]
  LIF scan tiles:        [h_part 128, t*(HT*B) + ht*B + b]
  readout y3 DRAM:       [O, T3, B]
"""

import sys
import numpy as np

try:
    import concourse.bass as bass
except ImportError:  # grading env fallback
    sys.path.insert(0, "/opt/trn_rl_repo")
    import concourse.bass as bass

import concourse.mybir as mybir
import concourse.tile as tile
from contextlib import ExitStack
from concourse import bacc
from concourse.masks import make_identity

F32 = mybir.dt.float32
F16 = mybir.dt.float16
AF = mybir.ActivationFunctionType
OP = mybir.AluOpType


class Cfg:
    def __init__(self, T0=300, B_loc=32, J=140, H=512, O=20, K=25, n_cores=8,
                 BETA=0.95, THRESH=1.0, SIG=0.5, EPS=1e-5, NT=16, CH=48,
                 CHUNK_TT=6):
        self.T0, self.B_loc, self.J, self.H, self.O, self.K = T0, B_loc, J, H, O, K
        self.n_cores = n_cores
        self.BETA, self.THRESH, self.SIG, self.EPS = BETA, THRESH, SIG, EPS
        self.LPAD, self.RPAD = K - 1, (K - 1) // 2
        self.PADT = self.LPAD + self.RPAD                      # 36
        self.T1 = T0 + self.RPAD                               # 312
        self.T2 = self.T1 + self.RPAD                          # 324
        self.T3 = self.T2 + self.RPAD                          # 336
        self.NT = NT                                           # out-steps per matmul tile
        self.CH = CH                                           # LIF chunk steps
        self.CHUNK_TT = CHUNK_TT                               # time-tiles per psum chunk
        self.HT = (H + 127) // 128                             # h tiles (4)
        self.B_tot = B_loc * n_cores
        self.J0 = min(J, 128)
        self.JL = J - self.J0                                  # leftover channels (12)
        self.HS = H // n_cores                                 # weight slab rows / core
        # packed weight matrix columns:
        #   w0 (H,J) | p0 (H,J) | w1T (in H, out H) | p1T | wrT (in H, O)
        #   | prT | g0 b0 g1 b1
        self.c_w0 = 0
        self.c_p0 = J
        self.c_w1 = 2 * J
        self.c_p1 = 2 * J + H
        self.c_wr = 2 * J + 2 * H
        self.c_pr = 2 * J + 2 * H + O
        self.c_gb = 2 * J + 2 * H + 2 * O
        self.WC = 2 * J + 2 * H + 2 * O + 4                    # 1348


def split_tiles(total, size):
    out = []
    t = 0
    while t < total:
        n = min(size, total - t)
        out.append((t, n))
        t += n
    return out


def bc(ap, axis, count):
    """Insert a stride-0 (broadcast) axis at position `axis` of an AP."""
    dims = [list(d) for d in ap.ap]
    dims.insert(axis, [0, count])
    return bass.AP(tensor=ap.tensor, offset=ap.offset, ap=dims)


def build_kernel(cfg: Cfg):
    c = cfg
    B, HT, K, H, O, J = c.B_loc, c.HT, c.K, c.H, c.O, c.J
    nc = bacc.Bacc("TRN2", target_bir_lowering=False, debug=False,
                   num_devices=c.n_cores)

    tts1 = split_tiles(c.T1, c.NT)
    tts2 = split_tiles(c.T2, c.NT)
    tts3 = split_tiles(c.T3, c.NT)
    n1slots = len(tts1)
    n2slots = len(tts2)

    # ---- I/O ----
    xh = nc.dram_tensor("xh", [B, c.T0, J], F32, kind="ExternalInput")
    wcs = nc.dram_tensor("wcs", [c.HS, c.WC], F32, kind="ExternalInput")
    out = nc.dram_tensor("out", [B, O], F32, kind="ExternalOutput")

    with tile.TileContext(nc) as tc, ExitStack() as ctx:
        dram = ctx.enter_context(tc.tile_pool(name="dram", bufs=1, space="DRAM"))
        cc_space = "Shared" if c.n_cores > 4 else "Local"
        wcf = dram.tile([H, c.WC], F32, name="wcf", addr_space=cc_space)
        y1d = dram.tile([HT, 128, c.T1, B], F32, name="y1d")
        s1d = dram.tile([HT, 128, c.T1 + c.PADT, B], F32, name="s1d")
        y2d = dram.tile([HT, 128, c.T2, B], F32, name="y2d")
        s2d = dram.tile([HT, 128, c.T2 + c.PADT, B], F32, name="s2d")
        y3d = dram.tile([O, c.T3, B], F32, name="y3d")
        k1dr = dram.tile([K, H, H], F32, name="k1dr")
        krdr = dram.tile([K, H, O], F32, name="krdr")
        cc1i = dram.tile([128, 2 * HT], F32, name="cc1i")
        cc1o = dram.tile([128, 2 * HT], F32, name="cc1o", addr_space=cc_space)
        cc2i = dram.tile([128, 2 * HT], F32, name="cc2i")
        cc2o = dram.tile([128, 2 * HT], F32, name="cc2o", addr_space=cc_space)

        # =============== Phase 0: AllGather packed weights ===============
        # (collectives cannot read IO tensors directly -> stage via DRAM)
        wci = dram.tile([c.HS, c.WC], F32, name="wci")
        nc.sync.dma_start(out=wci, in_=wcs.ap())
        nc.gpsimd.collective_compute(
            "AllGather", OP.bypass,
            replica_groups=[list(range(c.n_cores))],
            ins=[wci], outs=[wcf])

        glob = ctx.enter_context(tc.tile_pool(name="glob", bufs=1))

        # persistent small tiles
        sum1 = glob.tile([128, HT * n1slots], F32, name="sum1")
        sq1 = glob.tile([128, HT * n1slots], F32, name="sq1")
        sum2 = glob.tile([128, HT * n2slots], F32, name="sum2")
        sq2 = glob.tile([128, HT * n2slots], F32, name="sq2")
        gam0 = glob.tile([128, HT], F32, name="gam0")
        bet0 = glob.tile([128, HT], F32, name="bet0")
        gam1 = glob.tile([128, HT], F32, name="gam1")
        bet1 = glob.tile([128, HT], F32, name="bet1")
        for i, t in enumerate((gam0, bet0, gam1, bet1)):
            nc.sync.dma_start(
                out=t, in_=wcf[:, c.c_gb + i:c.c_gb + i + 1].rearrange(
                    "(ht p) o -> p (ht o)", p=128))
        A1 = glob.tile([128, HT], F32, name="A1")
        C1b = glob.tile([128, HT * B], F32, name="C1b")
        A2 = glob.tile([128, HT], F32, name="A2")
        C2b = glob.tile([128, HT * B], F32, name="C2b")
        zpad = glob.tile([128, c.LPAD * B], F32, name="zpad")
        nc.vector.memset(zpad, 0.0)

        # zero the pad regions of the spike dram buffers
        for sd, T in ((s1d, c.T1), (s2d, c.T2)):
            for ht in range(HT):
                nc.sync.dma_start(out=sd[ht, :, 0:c.LPAD, :],
                                  in_=zpad.rearrange("p (t b) -> p t b", b=B))
                nc.sync.dma_start(
                    out=sd[ht, :, T + c.LPAD:T + c.PADT, :],
                    in_=zpad.rearrange("p (t b) -> p t b", b=B)[:, :c.RPAD, :])

        # =============== Phase 0b: on-device DCLS kernels ===============
        # G[p,(k,m)] = exp(-2*(pT - (k-K//2))^2); S = sum_k G;
        # k_out = G * (wT / (S + 1e-7))
        def dcls_build(pool, kvf, wT, pT, jn, G, m, ktag):
            """Fill G[:jn] (layout [part, (k, m)]) with the DCLS kernel."""
            G3 = G.rearrange("p (k m) -> p k m", k=K)
            nc.vector.tensor_sub(G3[:jn], bc(pT[:jn], 1, K),
                                 bc(kvf[:jn], 2, m))
            nc.scalar.activation(out=G[:jn], in_=G[:jn], func=AF.Square)
            nc.scalar.activation(out=G[:jn], in_=G[:jn], func=AF.Exp,
                                 scale=-2.0)
            S = pool.tile([128, m], F32, tag=f"S{ktag}", name="S")
            nc.vector.reduce_sum(
                out=S[:jn],
                in_=G.rearrange("p (k m) -> p m k", k=K)[:jn],
                axis=mybir.AxisListType.X)
            nc.vector.tensor_scalar_add(S[:jn], S[:jn], 1e-7)
            nc.vector.reciprocal(S[:jn], S[:jn])
            nc.vector.tensor_mul(S[:jn], S[:jn], wT[:jn])   # S -> w / sum
            nc.vector.tensor_mul(G3[:jn], G3[:jn], bc(S[:jn], 1, K))

        with ExitStack() as p0:
            wpre = p0.enter_context(tc.tile_pool(name="wpre", bufs=1))
            kvf = wpre.tile([128, K], F32, name="kvf")
            nc.gpsimd.iota(kvf, pattern=[[1, K]], base=-(K // 2),
                           channel_multiplier=0,
                           allow_small_or_imprecise_dtypes=True)
            # layer-2 kernel k1dr [K, H(in), H(out)]
            for ct in range(HT):
                w1T = wpre.tile([128, H], F32, tag="w1T", name="w1T")
                nc.sync.dma_start(
                    out=w1T, in_=wcf[ct * 128:(ct + 1) * 128,
                                     c.c_w1:c.c_w1 + H])
                p1T = wpre.tile([128, H], F32, tag="p1T", name="p1T")
                nc.sync.dma_start(
                    out=p1T, in_=wcf[ct * 128:(ct + 1) * 128,
                                     c.c_p1:c.c_p1 + H])
                G = wpre.tile([128, K * H], F32, tag="G1", name="G1", bufs=1)
                dcls_build(wpre, kvf, w1T, p1T, 128, G, H, "1")
                nc.sync.dma_start(
                    out=k1dr[:, ct * 128:(ct + 1) * 128, :].rearrange(
                        "k p m -> p k m"),
                    in_=G.rearrange("p (k m) -> p k m", k=K))
            # readout kernel krdr [K, H(in), O]
            for ct in range(HT):
                wrT = wpre.tile([128, O], F32, tag="wrT", name="wrT")
                nc.sync.dma_start(
                    out=wrT, in_=wcf[ct * 128:(ct + 1) * 128,
                                     c.c_wr:c.c_wr + O])
                prT = wpre.tile([128, O], F32, tag="prT", name="prT")
                nc.sync.dma_start(
                    out=prT, in_=wcf[ct * 128:(ct + 1) * 128,
                                     c.c_pr:c.c_pr + O])
                Gr = wpre.tile([128, K * O], F32, tag="Gr", name="Gr")
                dcls_build(wpre, kvf, wrT, prT, 128, Gr, O, "r")
                nc.sync.dma_start(
                    out=krdr[:, ct * 128:(ct + 1) * 128, :].rearrange(
                        "k p m -> p k m"),
                    in_=Gr.rearrange("p (k m) -> p k m", k=K))

        # =============== Phase 1: conv1 (x -> y1) + stats ===============
        with ExitStack() as p1:
            psum = p1.enter_context(tc.tile_pool(name="psum1", bufs=8,
                                                  space="PSUM"))
            xpool = p1.enter_context(tc.tile_pool(name="xpool", bufs=1))
            wpool1 = p1.enter_context(tc.tile_pool(name="wpool1", bufs=1))
            stg1 = p1.enter_context(tc.tile_pool(name="stg1", bufs=3))

            # conv1 DCLS weights: W0 [J0, (k,h)], W1l [JL, (k,h)]
            W0 = wpool1.tile([c.J0, K * H], F32, name="W0")
            if c.JL:
                W1l = wpool1.tile([c.JL, K * H], F32, name="W1l")
            with ExitStack() as pw:
                wpre1 = pw.enter_context(tc.tile_pool(name="wpre1", bufs=1))
                kvf1 = wpre1.tile([128, K], F32, name="kvf1")
                nc.gpsimd.iota(kvf1, pattern=[[1, K]], base=-(K // 2),
                               channel_multiplier=0,
                               allow_small_or_imprecise_dtypes=True)
                jts = [(0, c.J0, W0)] + ([(c.J0, c.JL, W1l)] if c.JL else [])
                for (j0, jn, Wdst) in jts:
                    w0T = wpre1.tile([128, H], F32, tag="w0T", name="w0T")
                    nc.sync.dma_start(
                        out=w0T[:jn],
                        in_=wcf[:, c.c_w0 + j0:c.c_w0 + j0 + jn].rearrange(
                            "h p -> p h"))
                    p0T = wpre1.tile([128, H], F32, tag="p0T", name="p0T")
                    nc.sync.dma_start(
                        out=p0T[:jn],
                        in_=wcf[:, c.c_p0 + j0:c.c_p0 + j0 + jn].rearrange(
                            "h p -> p h"))
                    dcls_build(wpre1, kvf1, w0T, p0T, jn, Wdst, H, "0")

            # x in f16: [J0, (t,b)] padded in time
            T0p = c.T0 + c.PADT
            X0 = xpool.tile([c.J0, T0p * B], F32, name="X0")
            nc.vector.memset(X0, 0.0)
            X0v = X0.rearrange("p (t b) -> p t b", b=B)
            xhv = xh.ap().rearrange("b t j -> j b t")
            for b in range(B):
                nc.sync.dma_start(
                    out=X0v[:, c.LPAD:c.LPAD + c.T0, b],
                    in_=xhv[0:c.J0, b])
            if c.JL:
                X1 = xpool.tile([c.JL, T0p * B], F32, name="X1")
                nc.vector.memset(X1, 0.0)
                X1v = X1.rearrange("p (t b) -> p t b", b=B)
                for b in range(B):
                    nc.sync.dma_start(
                        out=X1v[:, c.LPAD:c.LPAD + c.T0, b],
                        in_=xhv[c.J0:J, b])

            n_mm = K * (2 if c.JL else 1)
            for tti, (t0, nt) in enumerate(tts1):
                for ht in range(HT):
                    ps = psum.tile([128, nt * B], F32, tag="cv1ps", name="ps1")
                    mi = 0
                    for kk in range(K):
                        nc.tensor.matmul(
                            ps, lhsT=W0[:, kk * H + ht * 128: kk * H + ht * 128 + 128],
                            rhs=X0[:, (t0 + kk) * B:(t0 + kk) * B + nt * B],
                            start=(mi == 0), stop=(mi == n_mm - 1))
                        mi += 1
                        if c.JL:
                            nc.tensor.matmul(
                                ps,
                                lhsT=W1l[:, kk * H + ht * 128: kk * H + ht * 128 + 128],
                                rhs=X1[:, (t0 + kk) * B:(t0 + kk) * B + nt * B],
                                start=(mi == 0), stop=(mi == n_mm - 1))
                            mi += 1
                    slot = ht * n1slots + tti
                    ystg = stg1.tile([128, nt * B], F32, tag="ystg", name="ystg")
                    nc.scalar.activation(out=ystg, in_=ps, func=AF.Copy,
                                         accum_out=sum1[:, slot:slot + 1])
                    ysq = stg1.tile([128, nt * B], F32, tag="ysq", name="ysq",
                                    bufs=1)
                    nc.scalar.activation(out=ysq, in_=ps, func=AF.Square,
                                         accum_out=sq1[:, slot:slot + 1])
                    nc.sync.dma_start(
                        out=y1d[ht, :, t0:t0 + nt, :],
                        in_=ystg.rearrange("p (t b) -> p t b", b=B))

        # =============== BN stats: allreduce + affine ===============
        def bn_affine(sumt, sqt, nslots, N, gam, bet, cci, cco, A, Cb, tagp):
            with ExitStack() as pb:
                sp = pb.enter_context(tc.tile_pool(name=f"bn{tagp}", bufs=1))
                ccs = sp.tile([128, 2 * HT], F32, name=f"ccs{tagp}")
                nc.vector.reduce_sum(
                    out=ccs[:, 0:HT],
                    in_=sumt.rearrange("p (h s) -> p h s", s=nslots),
                    axis=mybir.AxisListType.X)
                nc.vector.reduce_sum(
                    out=ccs[:, HT:2 * HT],
                    in_=sqt.rearrange("p (h s) -> p h s", s=nslots),
                    axis=mybir.AxisListType.X)
                nc.sync.dma_start(out=cci, in_=ccs)
                nc.gpsimd.collective_compute(
                    "AllReduce", OP.add,
                    replica_groups=[list(range(c.n_cores))],
                    ins=[cci], outs=[cco])
                gs = sp.tile([128, 2 * HT], F32, name=f"gs{tagp}")
                nc.sync.dma_start(out=gs, in_=cco)
                rN = float(1.0 / N)
                mu = sp.tile([128, HT], F32, name=f"mu{tagp}")
                nc.vector.tensor_scalar(mu, gs[:, 0:HT], rN, None, OP.mult)
                ex2 = sp.tile([128, HT], F32, name=f"ex2{tagp}")
                nc.vector.tensor_scalar(ex2, gs[:, HT:2 * HT], rN, None,
                                        OP.mult)
                var = sp.tile([128, HT], F32, name=f"var{tagp}")
                # var = ex2 - mu*mu ; then + eps
                nc.vector.scalar_tensor_tensor(out=var, in0=mu, scalar=1.0,
                                               in1=mu, op0=OP.mult, op1=OP.mult)
                nc.vector.tensor_sub(var, ex2, var)
                nc.vector.tensor_scalar_add(var, var, float(c.EPS))
                sv = sp.tile([128, HT], F32, name=f"sv{tagp}")
                nc.scalar.activation(out=sv, in_=var, func=AF.Sqrt)
                # one Newton step: s' = 0.5*(s + v/s)  (ACT sqrt is ~3e-6 approx)
                rs0 = sp.tile([128, HT], F32, name=f"rs0{tagp}")
                nc.vector.reciprocal(rs0, sv)
                t1 = sp.tile([128, HT], F32, name=f"t1{tagp}")
                nc.vector.tensor_mul(t1, var, rs0)
                nc.vector.tensor_add(sv, sv, t1)
                nc.vector.tensor_scalar(sv, sv, 0.5, None, OP.mult)
                rsv = sp.tile([128, HT], F32, name=f"rsv{tagp}")
                nc.vector.reciprocal(rsv, sv)
                nc.vector.tensor_mul(A, gam, rsv)
                # Cbias = bet - mu*A, broadcast over batch
                cb1 = sp.tile([128, HT], F32, name=f"cb1{tagp}")
                nc.vector.tensor_mul(cb1, mu, A)
                nc.vector.tensor_sub(cb1, bet, cb1)
                nc.vector.tensor_copy(
                    Cb.rearrange("p (h b) -> p h b", b=B), bc(cb1, 2, B))

        bn_affine(sum1, sq1, n1slots, c.T1 * c.B_tot, gam0, bet0,
                  cc1i, cc1o, A1, C1b, "1")

        # =============== LIF layer (generic) ===============
        def lif_layer(yd, sd, A, Cb, T, tag):
            with ExitStack() as pl:
                lp = pl.enter_context(tc.tile_pool(name=f"lif{tag}", bufs=2))
                up = pl.enter_context(tc.tile_pool(name=f"lifu{tag}", bufs=1))
                HTB = HT * B
                U = up.tile([128, HTB], F32, name=f"U{tag}")
                nc.vector.memset(U, 0.0)
                for (c0, cn) in split_tiles(T, c.CH):
                    ybufs = []
                    for ht in range(HT):
                        yb = lp.tile([128, cn * B], F32, tag=f"yb{ht}",
                                     name=f"yb{tag}")
                        nc.sync.dma_start(
                            out=yb.rearrange("p (t b) -> p t b", b=B),
                            in_=yd[ht, :, c0:c0 + cn, :])
                        ybufs.append(yb)
                    scn = lp.tile([128, cn * HTB], F32, tag="scn",
                                  name=f"scn{tag}")
                    scn3 = scn.rearrange("p (t x) -> p t x", x=HTB)
                    for ht in range(HT):
                        nc.vector.scalar_tensor_tensor(
                            out=scn3[:, :, ht * B:(ht + 1) * B],
                            in0=ybufs[ht].rearrange("p (t b) -> p t b", b=B),
                            scalar=A[:, ht:ht + 1],
                            in1=bc(Cb[:, ht * B:(ht + 1) * B], 1, cn),
                            op0=OP.mult, op1=OP.add)
                    S = lp.tile([128, cn * HTB], F32, tag="S", name=f"S{tag}")
                    for t in range(cn):
                        sl = slice(t * HTB, (t + 1) * HTB)
                        ut = lp.tile([128, HTB], F32, tag="ut", name=f"ut{tag}")
                        nc.vector.scalar_tensor_tensor(
                            out=ut, in0=U, scalar=float(c.BETA),
                            in1=scn[:, sl], op0=OP.mult, op1=OP.add)
                        nc.vector.tensor_scalar(
                            S[:, sl], ut, float(c.THRESH), None, OP.is_ge)
                        nc.vector.scalar_tensor_tensor(
                            out=U, in0=ut, scalar=float(c.THRESH), in1=ut,
                            op0=OP.is_lt, op1=OP.mult)
                    S3 = S.rearrange("p (t h b) -> p t h b", h=HT, b=B)
                    for ht in range(HT):
                        nc.sync.dma_start(
                            out=sd[ht, :, c.LPAD + c0:c.LPAD + c0 + cn, :],
                            in_=S3[:, :, ht, :])

        lif_layer(y1d, s1d, A1, C1b, c.T1, "1")

        # =============== conv from spikes (generic: layer 2 & readout) =====
        def conv_sp(sd, wsrc, M, tts, yd=None, sumt=None, sqt=None,
                    nslots=0, y3=None, tag=""):
            """y[o, t] = sum_{ct,k} W_k[ct]^T s[ct, t+k] (padded s)."""
            MT = (M + 127) // 128
            tchunks = split_tiles(len(tts), c.CHUNK_TT)
            with ExitStack() as pc:
                psum = pc.enter_context(tc.tile_pool(name=f"psum{tag}",
                                                     bufs=8, space="PSUM"))
                swp = pc.enter_context(tc.tile_pool(name=f"swin{tag}", bufs=2))
                wp = pc.enter_context(tc.tile_pool(name=f"w{tag}", bufs=3))
                sg = pc.enter_context(tc.tile_pool(name=f"stg{tag}", bufs=3))
                for (tci, ntt) in tchunks:
                    tt_group = tts[tci:tci + ntt]
                    w0_ = tt_group[0][0]
                    last_t0, last_nt = tt_group[-1]
                    winlen = (last_t0 + last_nt - 1 + K - 1) - w0_ + 1
                    swin = []
                    for ct in range(HT):
                        sw = swp.tile([128, winlen * B], F32, tag=f"sw{ct}",
                                      name=f"sw{tag}")
                        nc.sync.dma_start(
                            out=sw.rearrange("p (t b) -> p t b", b=B),
                            in_=sd[ct, :, w0_:w0_ + winlen, :])
                        swin.append(sw)
                    for ht in range(MT):
                        m0 = ht * 128
                        mtw = min(128, M - m0)
                        pss = [psum.tile([128, nt * B], F32, tag="cvps",
                                         name=f"ps{tag}")
                               for (t0, nt) in tt_group]
                        n_acc = HT * K
                        mi = 0
                        for ct in range(HT):
                            wt = wp.tile([128, K * mtw], F32, tag="wt",
                                         name=f"wt{tag}")
                            nc.sync.dma_start(
                                out=wt.rearrange("p (k m) -> p k m", m=mtw),
                                in_=wsrc[:, ct * 128:(ct + 1) * 128,
                                         m0:m0 + mtw].rearrange(
                                             "k p m -> p k m"))
                            for kk in range(K):
                                lhsT = wt[:, kk * mtw:(kk + 1) * mtw]
                                st = (mi == 0)
                                sp_ = (mi == n_acc - 1)
                                for ti, (t0, nt) in enumerate(tt_group):
                                    off = (t0 - w0_ + kk) * B
                                    nc.tensor.matmul(
                                        pss[ti][:mtw], lhsT=lhsT,
                                        rhs=swin[ct][:, off:off + nt * B],
                                        start=st, stop=sp_)
                                mi += 1
                        for ti, (t0, nt) in enumerate(tt_group):
                            stg = sg.tile([128, nt * B], F32, tag="stg",
                                          name=f"stg{tag}")
                            if sumt is not None:
                                slot = ht * nslots + tci + ti
                                nc.scalar.activation(
                                    out=stg[:mtw], in_=pss[ti][:mtw],
                                    func=AF.Copy,
                                    accum_out=sumt[:, slot:slot + 1])
                                sqg = sg.tile([128, nt * B], F32, tag="sqg",
                                              name=f"sqg{tag}")
                                nc.scalar.activation(
                                    out=sqg[:mtw], in_=pss[ti][:mtw],
                                    func=AF.Square,
                                    accum_out=sqt[:, slot:slot + 1])
                            else:
                                nc.scalar.activation(out=stg[:mtw],
                                                     in_=pss[ti][:mtw],
                                                     func=AF.Copy)
                            if yd is not None:
                                nc.sync.dma_start(
                                    out=yd[ht, :, t0:t0 + nt, :],
                                    in_=stg.rearrange("p (t b) -> p t b", b=B))
                            else:  # readout: y3 is [O, T3, B]
                                nc.sync.dma_start(
                                    out=y3[m0:m0 + mtw, t0:t0 + nt, :],
                                    in_=stg[:mtw].rearrange(
                                        "p (t b) -> p t b", b=B))

        conv_sp(s1d, k1dr, H, tts2, yd=y2d, sumt=sum2, sqt=sq2,
                nslots=n2slots, tag="c2")
        bn_affine(sum2, sq2, n2slots, c.T2 * c.B_tot, gam1, bet1,
                  cc2i, cc2o, A2, C2b, "2")
        lif_layer(y2d, s2d, A2, C2b, c.T2, "2")
        conv_sp(s2d, krdr, O, tts3, y3=y3d, tag="c3")

        # =============== tail: LI scan, softmax over O, sum over t =========
        with ExitStack() as pt:
            psum = pt.enter_context(tc.tile_pool(name="psumt", bufs=1,
                                                 space="PSUM"))
            tp = pt.enter_context(tc.tile_pool(name="tail", bufs=1))
            tp2 = pt.enter_context(tc.tile_pool(name="tail2", bufs=3))
            TB = c.T3 * B
            Y3 = tp.tile([O, TB], F32, name="Y3")
            nc.sync.dma_start(out=Y3.rearrange("p (t b) -> p t b", b=B),
                              in_=y3d)
            beta_t = tp.tile([128, c.T3], F32, name="beta_t")
            nc.vector.memset(beta_t, float(c.BETA))
            idn = tp.tile([128, 128], F32, name="idn")
            make_identity(nc, idn)
            # selb[p, b] = 1 iff p % B == b  (from identity: sum over r of
            # idn[p, r*B + b]); requires 128 % B == 0
            selbt = tp.tile([128, B], F32, name="selbt")
            nc.vector.reduce_sum(
                out=selbt, in_=idn.rearrange("p (r b) -> p b r", b=B),
                axis=mybir.AxisListType.X)
            us = tp.tile([O, TB], F32, name="us")
            # LI scan over t, one strided scan per batch column
            usv = us.rearrange("p (t b) -> p b t", b=B)
            y3v = Y3.rearrange("p (t b) -> p b t", b=B)
            for b in range(B):
                nc.vector.tensor_tensor_scan(
                    out=usv[:, b, :], data0=beta_t[:O], data1=y3v[:, b, :],
                    initial=0.0, op0=OP.mult, op1=OP.add)
            # per-128-col blocks: transpose to (t*b, o), softmax over o, then
            # sum over t via selector matmul into (B, O)
            acc = psum.tile([B, O], F32, tag="accps", name="accps", bufs=1)
            blocks = split_tiles(TB, 128)
            for bi, (c0, cw) in enumerate(blocks):
                pst = psum.tile([128, O], F32, tag="tpps", name="tpps", bufs=2)
                nc.tensor.transpose(out=pst[:cw, :O],
                                    in_=us[:, c0:c0 + cw],
                                    identity=idn[:O, :O])
                v = tp2.tile([128, O], F32, tag="v", name="v")
                nc.scalar.copy(out=v[:cw], in_=pst[:cw, :O])
                mx = tp2.tile([128, 1], F32, tag="mx", name="mx")
                nc.vector.reduce_max(out=mx[:cw], in_=v[:cw],
                                     axis=mybir.AxisListType.X)
                ev = tp2.tile([128, O], F32, tag="ev", name="ev")
                nc.vector.tensor_scalar(ev[:cw], v[:cw], mx[:cw], None,
                                        OP.subtract)
                pv = tp2.tile([128, O], F32, tag="pv", name="pv")
                sm = tp2.tile([128, 1], F32, tag="sm", name="sm")
                nc.scalar.activation(out=pv[:cw], in_=ev[:cw], func=AF.Exp,
                                     accum_out=sm[:cw])
                rsm = tp2.tile([128, 1], F32, tag="rsm", name="rsm")
                nc.vector.reciprocal(rsm[:cw], sm[:cw])
                pn_t = tp2.tile([128, O], F32, tag="pnt", name="pnt")
                nc.vector.tensor_scalar(pn_t[:cw], pv[:cw], rsm[:cw], None,
                                        OP.mult)
                nc.tensor.matmul(
                    acc, lhsT=selbt[:cw], rhs=pn_t[:cw],
                    start=(bi == 0), stop=(bi == len(blocks) - 1),
                    skip_group_check=True)
            res = tp.tile([B, O], F32, name="res")
            nc.scalar.copy(out=res, in_=acc)
            nc.sync.dma_start(out=out.ap(), in_=res)

    nc.compile()
    return nc


# ======================= host side =======================

def make_global_inputs(cfg: Cfg, x, w0, p0, g0, b0, w1, p1, g1, b1, wr, pr):
    """Pack the full inputs into the two global device arrays."""
    c = cfg
    xh = np.ascontiguousarray(np.asarray(x, np.float32))   # (B_tot, T0, J)
    wcat = np.empty((c.H, c.WC), np.float32)
    wcat[:, c.c_w0:c.c_w0 + c.J] = np.asarray(w0, np.float32)
    wcat[:, c.c_p0:c.c_p0 + c.J] = np.asarray(p0, np.float32)
    wcat[:, c.c_w1:c.c_w1 + c.H] = np.asarray(w1, np.float32).T
    wcat[:, c.c_p1:c.c_p1 + c.H] = np.asarray(p1, np.float32).T
    wcat[:, c.c_wr:c.c_wr + c.O] = np.asarray(wr, np.float32).T
    wcat[:, c.c_pr:c.c_pr + c.O] = np.asarray(pr, np.float32).T
    wcat[:, c.c_gb + 0] = np.asarray(g0, np.float32)
    wcat[:, c.c_gb + 1] = np.asarray(b0, np.float32)
    wcat[:, c.c_gb + 2] = np.asarray(g1, np.float32)
    wcat[:, c.c_gb + 3] = np.asarray(b1, np.float32)
    return {"xh": xh, "wcs": wcat}


def split_inputs(cfg: Cfg, gmap):
    """Per-core input maps (for the multi-core simulator)."""
    c = cfg
    maps = []
    for ci in range(c.n_cores):
        maps.append({
            "xh": gmap["xh"][ci * c.B_loc:(ci + 1) * c.B_loc],
            "wcs": gmap["wcs"][ci * c.HS:(ci + 1) * c.HS],
        })
    return maps


_EXEC_CACHE = {}


def _get_exec(cfg: Cfg):
    """Build (once) and cache a jitted sharded executable for this config."""
    key = (cfg.T0, cfg.B_loc, cfg.J, cfg.H, cfg.O, cfg.K, cfg.n_cores)
    if key in _EXEC_CACHE:
        return _EXEC_CACHE[key]

    import jax
    from jax.sharding import Mesh, PartitionSpec
    try:
        from jax.experimental.shard_map import shard_map
    except ImportError:
        from jax import shard_map
    from concourse.bass2jax import (_bass_exec_p, install_neuronx_cc_hook,
                                    partition_id_tensor)

    nc = build_kernel(cfg)
    install_neuronx_cc_hook()
    assert nc.dbg_addr is None, "kernel must be built with debug=False"

    partition_name = (nc.partition_id_tensor.name
                      if nc.partition_id_tensor else None)
    in_names, out_names, out_avals = [], [], []
    for alloc in nc.m.functions[0].allocations:
        if not isinstance(alloc, mybir.MemoryLocationSet):
            continue
        name = alloc.memorylocations[0].name
        if alloc.kind == "ExternalInput":
            if name != partition_name:
                in_names.append(name)
        elif alloc.kind == "ExternalOutput":
            out_names.append(name)
            out_avals.append(jax.core.ShapedArray(
                tuple(alloc.tensor_shape), mybir.dt.np(alloc.dtype)))
    n_params = len(in_names)
    n_outs = len(out_avals)
    all_in_names = (list(in_names) + out_names
                    + ([partition_name] if partition_name else []))

    def _body(*args):
        operands = list(args)
        if partition_name is not None:
            operands.append(partition_id_tensor())
        outs = _bass_exec_p.bind(
            *operands,
            out_avals=tuple(out_avals),
            in_names=tuple(all_in_names),
            out_names=tuple(out_names),
            lowering_input_output_aliases=(),
            sim_require_finite=True,
            sim_require_nnan=True,
            nc=nc,
        )
        return tuple(outs)

    devices = jax.devices()[:cfg.n_cores]
    assert len(devices) == cfg.n_cores
    mesh = Mesh(np.asarray(devices), ("core",))
    in_specs = (PartitionSpec("core"),) * (n_params + n_outs)
    out_specs = (PartitionSpec("core"),) * n_outs
    donate = tuple(range(n_params, n_params + n_outs))
    sharded = jax.jit(
        shard_map(_body, mesh=mesh, in_specs=in_specs, out_specs=out_specs,
                  check_rep=False),
        donate_argnums=donate, keep_unused=True)

    zero_shapes = [((cfg.n_cores * a.shape[0],) + tuple(a.shape[1:]), a.dtype)
                   for a in out_avals]

    def run_fn(gmap):
        args = [np.ascontiguousarray(gmap[nm]) for nm in in_names]
        zeros = [np.zeros(s, d) for (s, d) in zero_shapes]
        outs = sharded(*args, *zeros)
        return {nm: np.asarray(o) for nm, o in zip(out_names, outs)}

    _EXEC_CACHE[key] = run_fn
    return run_fn


def kernel(**inputs):
    cfg = Cfg()
    fn = _get_exec(cfg)
    gmap = make_global_inputs(cfg, **inputs)
    res = fn(gmap)
    return res["out"]


# revision 16
# speedup vs baseline: 1.5415x; 1.5415x over previous
"""Trainium2 Bass kernel for nn_DelayLIFSNN.

Architecture (per reference):
  x (B, T0, J) -> delay_conv(w0,p0) -> BN(global batch stats) -> LIF
               -> delay_conv(w1,p1) -> BN -> LIF
               -> delay_conv(wr,pr) -> LI readout -> sum_t softmax_o -> (B, O)

Sharding: data-parallel over batch B across 8 cores (B_loc=32/core);
BN stats all-reduced ((128, 2*HT) f32 = 4KB each).

Host<->device traffic is the wall-clock bottleneck (axon tunnel ~90MB/s),
so the host ships only:
  - x as float16 (B, T0, J)            (~21.5 MB total)
  - one packed weight matrix wcat [H, 1348] f32, SHARDED row-wise across
    cores (~2.7 MB total); an on-device AllGather reassembles it and the
    DCLS gaussian-interpolated conv kernels are computed on device.
The jitted PJRT executable is cached in a module global so repeat calls
don't re-trace.

Compute:
  Conv = sum over K=25 taps of shifted matmuls accumulated in PSUM
  (conv1 in f16, conv2/readout in f32).
  LIF = per-step scalar_tensor_tensor ops on DVE (sequential over time).
  LI readout = tensor_tensor_scan. Softmax+time-sum via PE transpose +
  selector-matmul.

Activation layouts:
  x / spikes (conv rhs): [ch_tile][ch_part 128, t*B + b]
  conv out psum:         [out_part 128, t*B + b] per (ht, time-tile)
  y DRAM:                [HT, 128, T, B]
  LIF scan tiles:        [h_part 128, t*(HT*B) + ht*B + b]
  readout y3 DRAM:       [O, T3, B]
"""

import sys
import numpy as np

try:
    import concourse.bass as bass
except ImportError:  # grading env fallback
    sys.path.insert(0, "/opt/trn_rl_repo")
    import concourse.bass as bass

import concourse.mybir as mybir
import concourse.tile as tile
from contextlib import ExitStack
from concourse import bacc
from concourse.masks import make_identity

F32 = mybir.dt.float32
F16 = mybir.dt.float16
AF = mybir.ActivationFunctionType
OP = mybir.AluOpType


class Cfg:
    def __init__(self, T0=300, B_loc=32, J=140, H=512, O=20, K=25, n_cores=8,
                 BETA=0.95, THRESH=1.0, SIG=0.5, EPS=1e-5, NT=16, CH=48,
                 CHUNK_TT=6):
        self.T0, self.B_loc, self.J, self.H, self.O, self.K = T0, B_loc, J, H, O, K
        self.n_cores = n_cores
        self.BETA, self.THRESH, self.SIG, self.EPS = BETA, THRESH, SIG, EPS
        self.LPAD, self.RPAD = K - 1, (K - 1) // 2
        self.PADT = self.LPAD + self.RPAD                      # 36
        self.T1 = T0 + self.RPAD                               # 312
        self.T2 = self.T1 + self.RPAD                          # 324
        self.T3 = self.T2 + self.RPAD                          # 336
        self.NT = NT                                           # out-steps per matmul tile
        self.CH = CH                                           # LIF chunk steps
        self.CHUNK_TT = CHUNK_TT                               # time-tiles per psum chunk
        self.HT = (H + 127) // 128                             # h tiles (4)
        self.B_tot = B_loc * n_cores
        self.J0 = min(J, 128)
        self.JL = J - self.J0                                  # leftover channels (12)
        self.HS = H // n_cores                                 # weight slab rows / core
        # packed weight matrix columns:
        #   w0 (H,J) | p0 (H,J) | w1T (in H, out H) | p1T | wrT (in H, O)
        #   | prT | g0 b0 g1 b1
        self.c_w0 = 0
        self.c_p0 = J
        self.c_w1 = 2 * J
        self.c_p1 = 2 * J + H
        self.c_wr = 2 * J + 2 * H
        self.c_pr = 2 * J + 2 * H + O
        self.c_gb = 2 * J + 2 * H + 2 * O
        self.WC = 2 * J + 2 * H + 2 * O + 4                    # 1348


def split_tiles(total, size):
    out = []
    t = 0
    while t < total:
        n = min(size, total - t)
        out.append((t, n))
        t += n
    return out


def bc(ap, axis, count):
    """Insert a stride-0 (broadcast) axis at position `axis` of an AP."""
    dims = [list(d) for d in ap.ap]
    dims.insert(axis, [0, count])
    return bass.AP(tensor=ap.tensor, offset=ap.offset, ap=dims)


def build_kernel(cfg: Cfg):
    c = cfg
    B, HT, K, H, O, J = c.B_loc, c.HT, c.K, c.H, c.O, c.J
    nc = bacc.Bacc("TRN2", target_bir_lowering=False, debug=False,
                   num_devices=c.n_cores)

    tts1 = split_tiles(c.T1, c.NT)
    tts2 = split_tiles(c.T2, c.NT)
    tts3 = split_tiles(c.T3, c.NT)
    n1slots = len(tts1)
    n2slots = len(tts2)

    # ---- I/O ----
    xh = nc.dram_tensor("xh", [B, c.T0, J], mybir.dt.uint16,
                    kind="ExternalInput")
    wcs = nc.dram_tensor("wcs", [c.HS, c.WC], F32, kind="ExternalInput")
    out = nc.dram_tensor("out", [B, O], F32, kind="ExternalOutput")

    with tile.TileContext(nc) as tc, ExitStack() as ctx:
        dram = ctx.enter_context(tc.tile_pool(name="dram", bufs=1, space="DRAM"))
        cc_space = "Shared" if c.n_cores > 4 else "Local"
        wcf = dram.tile([H, c.WC], F32, name="wcf", addr_space=cc_space)
        y1d = dram.tile([HT, 128, c.T1, B], F32, name="y1d")
        s1d = dram.tile([HT, 128, c.T1 + c.PADT, B], F32, name="s1d")
        y2d = dram.tile([HT, 128, c.T2, B], F32, name="y2d")
        s2d = dram.tile([HT, 128, c.T2 + c.PADT, B], F32, name="s2d")
        y3d = dram.tile([O, c.T3, B], F32, name="y3d")
        k1dr = dram.tile([K, H, H], F32, name="k1dr")
        krdr = dram.tile([K, H, O], F32, name="krdr")
        cc1i = dram.tile([128, 2 * HT], F32, name="cc1i")
        cc1o = dram.tile([128, 2 * HT], F32, name="cc1o", addr_space=cc_space)
        cc2i = dram.tile([128, 2 * HT], F32, name="cc2i")
        cc2o = dram.tile([128, 2 * HT], F32, name="cc2o", addr_space=cc_space)

        # =============== Phase 0: AllGather packed weights ===============
        # (collectives cannot read IO tensors directly -> stage via DRAM)
        wci = dram.tile([c.HS, c.WC], F32, name="wci")
        nc.sync.dma_start(out=wci, in_=wcs.ap())
        nc.gpsimd.collective_compute(
            "AllGather", OP.bypass,
            replica_groups=[list(range(c.n_cores))],
            ins=[wci], outs=[wcf])

        glob = ctx.enter_context(tc.tile_pool(name="glob", bufs=1))

        # persistent small tiles
        sum1 = glob.tile([128, HT * n1slots], F32, name="sum1")
        sq1 = glob.tile([128, HT * n1slots], F32, name="sq1")
        sum2 = glob.tile([128, HT * n2slots], F32, name="sum2")
        sq2 = glob.tile([128, HT * n2slots], F32, name="sq2")
        gam0 = glob.tile([128, HT], F32, name="gam0")
        bet0 = glob.tile([128, HT], F32, name="bet0")
        gam1 = glob.tile([128, HT], F32, name="gam1")
        bet1 = glob.tile([128, HT], F32, name="bet1")
        for i, t in enumerate((gam0, bet0, gam1, bet1)):
            nc.sync.dma_start(
                out=t, in_=wcf[:, c.c_gb + i:c.c_gb + i + 1].rearrange(
                    "(ht p) o -> p (ht o)", p=128))
        A1 = glob.tile([128, HT], F32, name="A1")
        C1b = glob.tile([128, HT * B], F32, name="C1b")
        A2 = glob.tile([128, HT], F32, name="A2")
        C2b = glob.tile([128, HT * B], F32, name="C2b")
        zpad = glob.tile([128, c.LPAD * B], F32, name="zpad")
        nc.vector.memset(zpad, 0.0)

        # zero the pad regions of the spike dram buffers
        for sd, T in ((s1d, c.T1), (s2d, c.T2)):
            for ht in range(HT):
                nc.sync.dma_start(out=sd[ht, :, 0:c.LPAD, :],
                                  in_=zpad.rearrange("p (t b) -> p t b", b=B))
                nc.sync.dma_start(
                    out=sd[ht, :, T + c.LPAD:T + c.PADT, :],
                    in_=zpad.rearrange("p (t b) -> p t b", b=B)[:, :c.RPAD, :])

        # =============== Phase 0b: on-device DCLS kernels ===============
        # G[p,(k,m)] = exp(-2*(pT - (k-K//2))^2); S = sum_k G;
        # k_out = G * (wT / (S + 1e-7))
        def dcls_build(pool, kvf, wT, pT, jn, G, m, ktag):
            """Fill G[:jn] (layout [part, (k, m)]) with the DCLS kernel."""
            G3 = G.rearrange("p (k m) -> p k m", k=K)
            nc.vector.tensor_sub(G3[:jn], bc(pT[:jn], 1, K),
                                 bc(kvf[:jn], 2, m))
            nc.scalar.activation(out=G[:jn], in_=G[:jn], func=AF.Square)
            nc.scalar.activation(out=G[:jn], in_=G[:jn], func=AF.Exp,
                                 scale=-2.0)
            S = pool.tile([128, m], F32, tag=f"S{ktag}", name="S")
            nc.vector.reduce_sum(
                out=S[:jn],
                in_=G.rearrange("p (k m) -> p m k", k=K)[:jn],
                axis=mybir.AxisListType.X)
            nc.vector.tensor_scalar_add(S[:jn], S[:jn], 1e-7)
            nc.vector.reciprocal(S[:jn], S[:jn])
            nc.vector.tensor_mul(S[:jn], S[:jn], wT[:jn])   # S -> w / sum
            nc.vector.tensor_mul(G3[:jn], G3[:jn], bc(S[:jn], 1, K))

        with ExitStack() as p0:
            wpre = p0.enter_context(tc.tile_pool(name="wpre", bufs=1))
            kvf = wpre.tile([128, K], F32, name="kvf")
            nc.gpsimd.iota(kvf, pattern=[[1, K]], base=-(K // 2),
                           channel_multiplier=0,
                           allow_small_or_imprecise_dtypes=True)
            # layer-2 kernel k1dr [K, H(in), H(out)]
            for ct in range(HT):
                w1T = wpre.tile([128, H], F32, tag="w1T", name="w1T")
                nc.sync.dma_start(
                    out=w1T, in_=wcf[ct * 128:(ct + 1) * 128,
                                     c.c_w1:c.c_w1 + H])
                p1T = wpre.tile([128, H], F32, tag="p1T", name="p1T")
                nc.sync.dma_start(
                    out=p1T, in_=wcf[ct * 128:(ct + 1) * 128,
                                     c.c_p1:c.c_p1 + H])
                G = wpre.tile([128, K * H], F32, tag="G1", name="G1", bufs=1)
                dcls_build(wpre, kvf, w1T, p1T, 128, G, H, "1")
                nc.sync.dma_start(
                    out=k1dr[:, ct * 128:(ct + 1) * 128, :].rearrange(
                        "k p m -> p k m"),
                    in_=G.rearrange("p (k m) -> p k m", k=K))
            # readout kernel krdr [K, H(in), O]
            for ct in range(HT):
                wrT = wpre.tile([128, O], F32, tag="wrT", name="wrT")
                nc.sync.dma_start(
                    out=wrT, in_=wcf[ct * 128:(ct + 1) * 128,
                                     c.c_wr:c.c_wr + O])
                prT = wpre.tile([128, O], F32, tag="prT", name="prT")
                nc.sync.dma_start(
                    out=prT, in_=wcf[ct * 128:(ct + 1) * 128,
                                     c.c_pr:c.c_pr + O])
                Gr = wpre.tile([128, K * O], F32, tag="Gr", name="Gr")
                dcls_build(wpre, kvf, wrT, prT, 128, Gr, O, "r")
                nc.sync.dma_start(
                    out=krdr[:, ct * 128:(ct + 1) * 128, :].rearrange(
                        "k p m -> p k m"),
                    in_=Gr.rearrange("p (k m) -> p k m", k=K))

        # =============== Phase 1: conv1 (x -> y1) + stats ===============
        with ExitStack() as p1:
            psum = p1.enter_context(tc.tile_pool(name="psum1", bufs=8,
                                                  space="PSUM"))
            xpool = p1.enter_context(tc.tile_pool(name="xpool", bufs=1))
            wpool1 = p1.enter_context(tc.tile_pool(name="wpool1", bufs=1))
            stg1 = p1.enter_context(tc.tile_pool(name="stg1", bufs=3))

            # x arrives as uint16 (q = round(x * 65536)), kept resident in
            # SBUF as u16; dequantized per time-tile into a small rotating
            # f32 scratch with the exact power-of-2 scale 2^-16
            T0p = c.T0 + c.PADT
            X0u = xpool.tile([c.J0, T0p * B], mybir.dt.uint16, name="X0u")
            nc.vector.memset(X0u, 0)
            X0uv = X0u.rearrange("p (t b) -> p t b", b=B)
            xhv = xh.ap().rearrange("b t j -> j b t")
            for b in range(B):
                nc.sync.dma_start(out=X0uv[:, c.LPAD:c.LPAD + c.T0, b],
                                  in_=xhv[0:c.J0, b])
            if c.JL:
                X1u = xpool.tile([c.JL, T0p * B], mybir.dt.uint16, name="X1u")
                nc.vector.memset(X1u, 0)
                X1uv = X1u.rearrange("p (t b) -> p t b", b=B)
                for b in range(B):
                    nc.sync.dma_start(out=X1uv[:, c.LPAD:c.LPAD + c.T0, b],
                                      in_=xhv[c.J0:J, b])

            # conv1 DCLS weights: W0 [J0, (k,h)], W1l [JL, (k,h)]
            W0 = wpool1.tile([c.J0, K * H], F32, name="W0")
            if c.JL:
                W1l = wpool1.tile([c.JL, K * H], F32, name="W1l")
            with ExitStack() as pw:
                wpre1 = pw.enter_context(tc.tile_pool(name="wpre1", bufs=1))
                kvf1 = wpre1.tile([128, K], F32, name="kvf1")
                nc.gpsimd.iota(kvf1, pattern=[[1, K]], base=-(K // 2),
                               channel_multiplier=0,
                               allow_small_or_imprecise_dtypes=True)
                jts = [(0, c.J0, W0)] + ([(c.J0, c.JL, W1l)] if c.JL else [])
                for (j0, jn, Wdst) in jts:
                    w0T = wpre1.tile([128, H], F32, tag="w0T", name="w0T")
                    nc.sync.dma_start(
                        out=w0T[:jn],
                        in_=wcf[:, c.c_w0 + j0:c.c_w0 + j0 + jn].rearrange(
                            "h p -> p h"))
                    p0T = wpre1.tile([128, H], F32, tag="p0T", name="p0T")
                    nc.sync.dma_start(
                        out=p0T[:jn],
                        in_=wcf[:, c.c_p0 + j0:c.c_p0 + j0 + jn].rearrange(
                            "h p -> p h"))
                    dcls_build(wpre1, kvf1, w0T, p0T, jn, Wdst, H, "0")


            n_mm = K * (2 if c.JL else 1)
            cvt = p1.enter_context(tc.tile_pool(name="cvt1", bufs=2))
            winb = (c.NT + K - 1) * B
            for tti, (t0, nt) in enumerate(tts1):
                ww = (nt + K - 1) * B
                XC0 = cvt.tile([c.J0, winb], F32, tag="xc0", name="XC0")
                nc.scalar.activation(out=XC0[:, :ww],
                                     in_=X0u[:, t0 * B:t0 * B + ww],
                                     func=AF.Copy, scale=float(2.0 ** -16))
                if c.JL:
                    XC1 = cvt.tile([c.JL, winb], F32, tag="xc1", name="XC1")
                    nc.scalar.activation(out=XC1[:, :ww],
                                         in_=X1u[:, t0 * B:t0 * B + ww],
                                         func=AF.Copy, scale=float(2.0 ** -16))
                for ht in range(HT):
                    ps = psum.tile([128, nt * B], F32, tag="cv1ps", name="ps1")
                    mi = 0
                    for kk in range(K):
                        nc.tensor.matmul(
                            ps, lhsT=W0[:, kk * H + ht * 128: kk * H + ht * 128 + 128],
                            rhs=XC0[:, kk * B:kk * B + nt * B],
                            start=(mi == 0), stop=(mi == n_mm - 1))
                        mi += 1
                        if c.JL:
                            nc.tensor.matmul(
                                ps,
                                lhsT=W1l[:, kk * H + ht * 128: kk * H + ht * 128 + 128],
                                rhs=XC1[:, kk * B:kk * B + nt * B],
                                start=(mi == 0), stop=(mi == n_mm - 1))
                            mi += 1
                    slot = ht * n1slots + tti
                    ystg = stg1.tile([128, nt * B], F32, tag="ystg", name="ystg")
                    nc.scalar.activation(out=ystg, in_=ps, func=AF.Copy,
                                         accum_out=sum1[:, slot:slot + 1])
                    ysq = stg1.tile([128, nt * B], F32, tag="ysq", name="ysq",
                                    bufs=1)
                    nc.scalar.activation(out=ysq, in_=ps, func=AF.Square,
                                         accum_out=sq1[:, slot:slot + 1])
                    nc.sync.dma_start(
                        out=y1d[ht, :, t0:t0 + nt, :],
                        in_=ystg.rearrange("p (t b) -> p t b", b=B))

        # =============== BN stats: allreduce + affine ===============
        def bn_affine(sumt, sqt, nslots, N, gam, bet, cci, cco, A, Cb, tagp):
            with ExitStack() as pb:
                sp = pb.enter_context(tc.tile_pool(name=f"bn{tagp}", bufs=1))
                ccs = sp.tile([128, 2 * HT], F32, name=f"ccs{tagp}")
                nc.vector.reduce_sum(
                    out=ccs[:, 0:HT],
                    in_=sumt.rearrange("p (h s) -> p h s", s=nslots),
                    axis=mybir.AxisListType.X)
                nc.vector.reduce_sum(
                    out=ccs[:, HT:2 * HT],
                    in_=sqt.rearrange("p (h s) -> p h s", s=nslots),
                    axis=mybir.AxisListType.X)
                nc.sync.dma_start(out=cci, in_=ccs)
                nc.gpsimd.collective_compute(
                    "AllReduce", OP.add,
                    replica_groups=[list(range(c.n_cores))],
                    ins=[cci], outs=[cco])
                gs = sp.tile([128, 2 * HT], F32, name=f"gs{tagp}")
                nc.sync.dma_start(out=gs, in_=cco)
                rN = float(1.0 / N)
                mu = sp.tile([128, HT], F32, name=f"mu{tagp}")
                nc.vector.tensor_scalar(mu, gs[:, 0:HT], rN, None, OP.mult)
                ex2 = sp.tile([128, HT], F32, name=f"ex2{tagp}")
                nc.vector.tensor_scalar(ex2, gs[:, HT:2 * HT], rN, None,
                                        OP.mult)
                var = sp.tile([128, HT], F32, name=f"var{tagp}")
                # var = ex2 - mu*mu ; then + eps
                nc.vector.scalar_tensor_tensor(out=var, in0=mu, scalar=1.0,
                                               in1=mu, op0=OP.mult, op1=OP.mult)
                nc.vector.tensor_sub(var, ex2, var)
                nc.vector.tensor_scalar_add(var, var, float(c.EPS))
                sv = sp.tile([128, HT], F32, name=f"sv{tagp}")
                nc.scalar.activation(out=sv, in_=var, func=AF.Sqrt)
                # one Newton step: s' = 0.5*(s + v/s)  (ACT sqrt is ~3e-6 approx)
                rs0 = sp.tile([128, HT], F32, name=f"rs0{tagp}")
                nc.vector.reciprocal(rs0, sv)
                t1 = sp.tile([128, HT], F32, name=f"t1{tagp}")
                nc.vector.tensor_mul(t1, var, rs0)
                nc.vector.tensor_add(sv, sv, t1)
                nc.vector.tensor_scalar(sv, sv, 0.5, None, OP.mult)
                rsv = sp.tile([128, HT], F32, name=f"rsv{tagp}")
                nc.vector.reciprocal(rsv, sv)
                nc.vector.tensor_mul(A, gam, rsv)
                # Cbias = bet - mu*A, broadcast over batch
                cb1 = sp.tile([128, HT], F32, name=f"cb1{tagp}")
                nc.vector.tensor_mul(cb1, mu, A)
                nc.vector.tensor_sub(cb1, bet, cb1)
                nc.vector.tensor_copy(
                    Cb.rearrange("p (h b) -> p h b", b=B), bc(cb1, 2, B))

        bn_affine(sum1, sq1, n1slots, c.T1 * c.B_tot, gam0, bet0,
                  cc1i, cc1o, A1, C1b, "1")

        # =============== LIF layer (generic) ===============
        def lif_layer(yd, sd, A, Cb, T, tag):
            with ExitStack() as pl:
                lp = pl.enter_context(tc.tile_pool(name=f"lif{tag}", bufs=2))
                up = pl.enter_context(tc.tile_pool(name=f"lifu{tag}", bufs=1))
                HTB = HT * B
                U = up.tile([128, HTB], F32, name=f"U{tag}")
                nc.vector.memset(U, 0.0)
                for (c0, cn) in split_tiles(T, c.CH):
                    ybufs = []
                    for ht in range(HT):
                        yb = lp.tile([128, cn * B], F32, tag=f"yb{ht}",
                                     name=f"yb{tag}")
                        nc.sync.dma_start(
                            out=yb.rearrange("p (t b) -> p t b", b=B),
                            in_=yd[ht, :, c0:c0 + cn, :])
                        ybufs.append(yb)
                    scn = lp.tile([128, cn * HTB], F32, tag="scn",
                                  name=f"scn{tag}")
                    scn3 = scn.rearrange("p (t x) -> p t x", x=HTB)
                    for ht in range(HT):
                        nc.vector.scalar_tensor_tensor(
                            out=scn3[:, :, ht * B:(ht + 1) * B],
                            in0=ybufs[ht].rearrange("p (t b) -> p t b", b=B),
                            scalar=A[:, ht:ht + 1],
                            in1=bc(Cb[:, ht * B:(ht + 1) * B], 1, cn),
                            op0=OP.mult, op1=OP.add)
                    S = lp.tile([128, cn * HTB], F32, tag="S", name=f"S{tag}")
                    for t in range(cn):
                        sl = slice(t * HTB, (t + 1) * HTB)
                        ut = lp.tile([128, HTB], F32, tag="ut", name=f"ut{tag}")
                        nc.vector.scalar_tensor_tensor(
                            out=ut, in0=U, scalar=float(c.BETA),
                            in1=scn[:, sl], op0=OP.mult, op1=OP.add)
                        nc.vector.tensor_scalar(
                            S[:, sl], ut, float(c.THRESH), None, OP.is_ge)
                        nc.vector.scalar_tensor_tensor(
                            out=U, in0=ut, scalar=float(c.THRESH), in1=ut,
                            op0=OP.is_lt, op1=OP.mult)
                    S3 = S.rearrange("p (t h b) -> p t h b", h=HT, b=B)
                    for ht in range(HT):
                        nc.sync.dma_start(
                            out=sd[ht, :, c.LPAD + c0:c.LPAD + c0 + cn, :],
                            in_=S3[:, :, ht, :])

        lif_layer(y1d, s1d, A1, C1b, c.T1, "1")

        # =============== conv from spikes (generic: layer 2 & readout) =====
        def conv_sp(sd, wsrc, M, tts, yd=None, sumt=None, sqt=None,
                    nslots=0, y3=None, tag=""):
            """y[o, t] = sum_{ct,k} W_k[ct]^T s[ct, t+k] (padded s)."""
            MT = (M + 127) // 128
            tchunks = split_tiles(len(tts), c.CHUNK_TT)
            with ExitStack() as pc:
                psum = pc.enter_context(tc.tile_pool(name=f"psum{tag}",
                                                     bufs=8, space="PSUM"))
                swp = pc.enter_context(tc.tile_pool(name=f"swin{tag}", bufs=2))
                wp = pc.enter_context(tc.tile_pool(name=f"w{tag}", bufs=3))
                sg = pc.enter_context(tc.tile_pool(name=f"stg{tag}", bufs=3))
                for (tci, ntt) in tchunks:
                    tt_group = tts[tci:tci + ntt]
                    w0_ = tt_group[0][0]
                    last_t0, last_nt = tt_group[-1]
                    winlen = (last_t0 + last_nt - 1 + K - 1) - w0_ + 1
                    swin = []
                    for ct in range(HT):
                        sw = swp.tile([128, winlen * B], F32, tag=f"sw{ct}",
                                      name=f"sw{tag}")
                        nc.sync.dma_start(
                            out=sw.rearrange("p (t b) -> p t b", b=B),
                            in_=sd[ct, :, w0_:w0_ + winlen, :])
                        swin.append(sw)
                    for ht in range(MT):
                        m0 = ht * 128
                        mtw = min(128, M - m0)
                        pss = [psum.tile([128, nt * B], F32, tag="cvps",
                                         name=f"ps{tag}")
                               for (t0, nt) in tt_group]
                        n_acc = HT * K
                        mi = 0
                        for ct in range(HT):
                            wt = wp.tile([128, K * mtw], F32, tag="wt",
                                         name=f"wt{tag}")
                            nc.sync.dma_start(
                                out=wt.rearrange("p (k m) -> p k m", m=mtw),
                                in_=wsrc[:, ct * 128:(ct + 1) * 128,
                                         m0:m0 + mtw].rearrange(
                                             "k p m -> p k m"))
                            for kk in range(K):
                                lhsT = wt[:, kk * mtw:(kk + 1) * mtw]
                                st = (mi == 0)
                                sp_ = (mi == n_acc - 1)
                                for ti, (t0, nt) in enumerate(tt_group):
                                    off = (t0 - w0_ + kk) * B
                                    nc.tensor.matmul(
                                        pss[ti][:mtw], lhsT=lhsT,
                                        rhs=swin[ct][:, off:off + nt * B],
                                        start=st, stop=sp_)
                                mi += 1
                        for ti, (t0, nt) in enumerate(tt_group):
                            stg = sg.tile([128, nt * B], F32, tag="stg",
                                          name=f"stg{tag}")
                            if sumt is not None:
                                slot = ht * nslots + tci + ti
                                nc.scalar.activation(
                                    out=stg[:mtw], in_=pss[ti][:mtw],
                                    func=AF.Copy,
                                    accum_out=sumt[:, slot:slot + 1])
                                sqg = sg.tile([128, nt * B], F32, tag="sqg",
                                              name=f"sqg{tag}")
                                nc.scalar.activation(
                                    out=sqg[:mtw], in_=pss[ti][:mtw],
                                    func=AF.Square,
                                    accum_out=sqt[:, slot:slot + 1])
                            else:
                                nc.scalar.activation(out=stg[:mtw],
                                                     in_=pss[ti][:mtw],
                                                     func=AF.Copy)
                            if yd is not None:
                                nc.sync.dma_start(
                                    out=yd[ht, :, t0:t0 + nt, :],
                                    in_=stg.rearrange("p (t b) -> p t b", b=B))
                            else:  # readout: y3 is [O, T3, B]
                                nc.sync.dma_start(
                                    out=y3[m0:m0 + mtw, t0:t0 + nt, :],
                                    in_=stg[:mtw].rearrange(
                                        "p (t b) -> p t b", b=B))

        conv_sp(s1d, k1dr, H, tts2, yd=y2d, sumt=sum2, sqt=sq2,
                nslots=n2slots, tag="c2")
        bn_affine(sum2, sq2, n2slots, c.T2 * c.B_tot, gam1, bet1,
                  cc2i, cc2o, A2, C2b, "2")
        lif_layer(y2d, s2d, A2, C2b, c.T2, "2")
        conv_sp(s2d, krdr, O, tts3, y3=y3d, tag="c3")

        # =============== tail: LI scan, softmax over O, sum over t =========
        with ExitStack() as pt:
            psum = pt.enter_context(tc.tile_pool(name="psumt", bufs=1,
                                                 space="PSUM"))
            tp = pt.enter_context(tc.tile_pool(name="tail", bufs=1))
            tp2 = pt.enter_context(tc.tile_pool(name="tail2", bufs=3))
            TB = c.T3 * B
            Y3 = tp.tile([O, TB], F32, name="Y3")
            nc.sync.dma_start(out=Y3.rearrange("p (t b) -> p t b", b=B),
                              in_=y3d)
            beta_t = tp.tile([128, c.T3], F32, name="beta_t")
            nc.vector.memset(beta_t, float(c.BETA))
            idn = tp.tile([128, 128], F32, name="idn")
            make_identity(nc, idn)
            # selb[p, b] = 1 iff p % B == b  (from identity: sum over r of
            # idn[p, r*B + b]); requires 128 % B == 0
            selbt = tp.tile([128, B], F32, name="selbt")
            nc.vector.reduce_sum(
                out=selbt, in_=idn.rearrange("p (r b) -> p b r", b=B),
                axis=mybir.AxisListType.X)
            us = tp.tile([O, TB], F32, name="us")
            # LI scan over t, one strided scan per batch column
            usv = us.rearrange("p (t b) -> p b t", b=B)
            y3v = Y3.rearrange("p (t b) -> p b t", b=B)
            for b in range(B):
                nc.vector.tensor_tensor_scan(
                    out=usv[:, b, :], data0=beta_t[:O], data1=y3v[:, b, :],
                    initial=0.0, op0=OP.mult, op1=OP.add)
            # per-128-col blocks: transpose to (t*b, o), softmax over o, then
            # sum over t via selector matmul into (B, O)
            acc = psum.tile([B, O], F32, tag="accps", name="accps", bufs=1)
            blocks = split_tiles(TB, 128)
            for bi, (c0, cw) in enumerate(blocks):
                pst = psum.tile([128, O], F32, tag="tpps", name="tpps", bufs=2)
                nc.tensor.transpose(out=pst[:cw, :O],
                                    in_=us[:, c0:c0 + cw],
                                    identity=idn[:O, :O])
                v = tp2.tile([128, O], F32, tag="v", name="v")
                nc.scalar.copy(out=v[:cw], in_=pst[:cw, :O])
                mx = tp2.tile([128, 1], F32, tag="mx", name="mx")
                nc.vector.reduce_max(out=mx[:cw], in_=v[:cw],
                                     axis=mybir.AxisListType.X)
                ev = tp2.tile([128, O], F32, tag="ev", name="ev")
                nc.vector.tensor_scalar(ev[:cw], v[:cw], mx[:cw], None,
                                        OP.subtract)
                pv = tp2.tile([128, O], F32, tag="pv", name="pv")
                sm = tp2.tile([128, 1], F32, tag="sm", name="sm")
                nc.scalar.activation(out=pv[:cw], in_=ev[:cw], func=AF.Exp,
                                     accum_out=sm[:cw])
                rsm = tp2.tile([128, 1], F32, tag="rsm", name="rsm")
                nc.vector.reciprocal(rsm[:cw], sm[:cw])
                pn_t = tp2.tile([128, O], F32, tag="pnt", name="pnt")
                nc.vector.tensor_scalar(pn_t[:cw], pv[:cw], rsm[:cw], None,
                                        OP.mult)
                nc.tensor.matmul(
                    acc, lhsT=selbt[:cw], rhs=pn_t[:cw],
                    start=(bi == 0), stop=(bi == len(blocks) - 1),
                    skip_group_check=True)
            res = tp.tile([B, O], F32, name="res")
            nc.scalar.copy(out=res, in_=acc)
            nc.sync.dma_start(out=out.ap(), in_=res)

    nc.compile()
    return nc


# ======================= host side =======================

def make_global_inputs(cfg: Cfg, x, w0, p0, g0, b0, w1, p1, g1, b1, wr, pr):
    """Pack the full inputs into the two global device arrays."""
    c = cfg
    xf = np.asarray(x, np.float32)
    xq = xf * np.float32(65536.0)
    xq += np.float32(0.5)
    np.minimum(xq, np.float32(65535.0), out=xq)
    xh = xq.astype(np.uint16)                              # (B_tot, T0, J)
    wcat = np.empty((c.H, c.WC), np.float32)
    wcat[:, c.c_w0:c.c_w0 + c.J] = np.asarray(w0, np.float32)
    wcat[:, c.c_p0:c.c_p0 + c.J] = np.asarray(p0, np.float32)
    wcat[:, c.c_w1:c.c_w1 + c.H] = np.asarray(w1, np.float32).T
    wcat[:, c.c_p1:c.c_p1 + c.H] = np.asarray(p1, np.float32).T
    wcat[:, c.c_wr:c.c_wr + c.O] = np.asarray(wr, np.float32).T
    wcat[:, c.c_pr:c.c_pr + c.O] = np.asarray(pr, np.float32).T
    wcat[:, c.c_gb + 0] = np.asarray(g0, np.float32)
    wcat[:, c.c_gb + 1] = np.asarray(b0, np.float32)
    wcat[:, c.c_gb + 2] = np.asarray(g1, np.float32)
    wcat[:, c.c_gb + 3] = np.asarray(b1, np.float32)
    return {"xh": xh, "wcs": wcat}


def split_inputs(cfg: Cfg, gmap):
    """Per-core input maps (for the multi-core simulator)."""
    c = cfg
    maps = []
    for ci in range(c.n_cores):
        maps.append({
            "xh": gmap["xh"][ci * c.B_loc:(ci + 1) * c.B_loc],
            "wcs": gmap["wcs"][ci * c.HS:(ci + 1) * c.HS],
        })
    return maps


_EXEC_CACHE = {}


def _get_exec(cfg: Cfg):
    """Build (once) and cache a jitted sharded executable for this config."""
    key = (cfg.T0, cfg.B_loc, cfg.J, cfg.H, cfg.O, cfg.K, cfg.n_cores)
    if key in _EXEC_CACHE:
        return _EXEC_CACHE[key]

    import jax
    from jax.sharding import Mesh, PartitionSpec
    try:
        from jax.experimental.shard_map import shard_map
    except ImportError:
        from jax import shard_map
    from concourse.bass2jax import (_bass_exec_p, install_neuronx_cc_hook,
                                    partition_id_tensor)

    nc = build_kernel(cfg)
    install_neuronx_cc_hook()
    assert nc.dbg_addr is None, "kernel must be built with debug=False"

    partition_name = (nc.partition_id_tensor.name
                      if nc.partition_id_tensor else None)
    in_names, out_names, out_avals = [], [], []
    for alloc in nc.m.functions[0].allocations:
        if not isinstance(alloc, mybir.MemoryLocationSet):
            continue
        name = alloc.memorylocations[0].name
        if alloc.kind == "ExternalInput":
            if name != partition_name:
                in_names.append(name)
        elif alloc.kind == "ExternalOutput":
            out_names.append(name)
            out_avals.append(jax.core.ShapedArray(
                tuple(alloc.tensor_shape), mybir.dt.np(alloc.dtype)))
    n_params = len(in_names)
    n_outs = len(out_avals)
    all_in_names = (list(in_names) + out_names
                    + ([partition_name] if partition_name else []))

    def _body(*args):
        operands = list(args)
        if partition_name is not None:
            operands.append(partition_id_tensor())
        outs = _bass_exec_p.bind(
            *operands,
            out_avals=tuple(out_avals),
            in_names=tuple(all_in_names),
            out_names=tuple(out_names),
            lowering_input_output_aliases=(),
            sim_require_finite=True,
            sim_require_nnan=True,
            nc=nc,
        )
        return tuple(outs)

    devices = jax.devices()[:cfg.n_cores]
    assert len(devices) == cfg.n_cores
    mesh = Mesh(np.asarray(devices), ("core",))
    in_specs = (PartitionSpec("core"),) * (n_params + n_outs)
    out_specs = (PartitionSpec("core"),) * n_outs
    donate = tuple(range(n_params, n_params + n_outs))
    sharded = jax.jit(
        shard_map(_body, mesh=mesh, in_specs=in_specs, out_specs=out_specs,
                  check_rep=False),
        donate_argnums=donate, keep_unused=True)

    zero_shapes = [((cfg.n_cores * a.shape[0],) + tuple(a.shape[1:]), a.dtype)
                   for a in out_avals]

    def run_fn(gmap):
        args = [np.ascontiguousarray(gmap[nm]) for nm in in_names]
        zeros = [np.zeros(s, d) for (s, d) in zero_shapes]
        outs = sharded(*args, *zeros)
        return {nm: np.asarray(o) for nm, o in zip(out_names, outs)}

    run_fn.mesh = mesh
    run_fn.devices = devices
    run_fn.in_names = in_names
    _EXEC_CACHE[key] = run_fn
    return run_fn


def kernel(**inputs):
    cfg = Cfg()
    fn = _get_exec(cfg)
    gmap = make_global_inputs(cfg, **inputs)
    res = fn(gmap)
    return res["out"]
